# revision 39
# baseline (speedup 1.0000x reference)
"""Trainium2 Bass kernel for windowed 3D attention with decomposed rel-pos bias.

Problem: B=4, N=4096 (16^3), C=384, window 8^3=512 tokens, 6 heads x 64 dim.
Sharding: 8 cores, data-parallel over 32 windows (4 per core). Core i takes
batch b=i//2, z-half h=i%2 -> a contiguous [2048, 384] slice of x holding 4
windows (wy, wx in {0,1}).

Device-side per window:
  xT via DMA-transpose; qT/kT produced in a "gapped" 88-row channel layout
  (gaps at rows 0-8/32-40/64-72 hold rel-pos terms / E-indicators so the
  S^T matmul contracts value+bias in ONE pass); S^T = stk^T @ rhsq in PSUM;
  exp on ACT -> P^T bf16; attn@V with a ones-column for the softmax
  denominator; normalize via DMA-broadcast denom + DVE divide; per-head
  projection accumulated in PSUM; bias via rank-1 matmuls.
"""

import os
import numpy as np
import ml_dtypes

BF16 = np.float16

B, N, C = 4, 4096, 384
WS, NH, HD = 8, 6, 64
T = WS * WS * WS  # 512
SCALE = HD ** -0.5
NCORES = 8

# gapped channel layout: 88 contraction rows per head
GAP_SLOTS = [0, 32, 64]                      # x, y, z rel slots (8 rows each)
CHAN_ROWS = list(range(8, 32)) + list(range(40, 64)) + list(range(72, 88))
GROW = 88

_CACHE = {}


def _build_nc():
    import concourse.bass as bass
    import concourse.tile as tile
    import concourse.mybir as mybir
    from contextlib import ExitStack

    f32 = mybir.dt.float32
    bf16 = mybir.dt.float16
    Ident = mybir.ActivationFunctionType.Identity
    Exp = mybir.ActivationFunctionType.Exp
    add = mybir.AluOpType.add
    divide = mybir.AluOpType.divide

    nc = bass.Bass("TRN2")

    x_d = nc.declare_dram_parameter("xt_sh", [4, 128, 3, T], bf16, isOutput=False)
    wq_d = nc.declare_dram_parameter("wq_g", [128, NH * 3 * GROW], bf16, isOutput=False)
    wk_d = nc.declare_dram_parameter("wk_g", [128, NH * 3 * GROW], bf16, isOutput=False)
    wv_d = nc.declare_dram_parameter("wv", [128, 3 * 390], bf16, isOutput=False)
    bq_d = nc.declare_dram_parameter("bq_g", [128, NH], f32, isOutput=False)
    bk_d = nc.declare_dram_parameter("bk_g", [128, NH], f32, isOutput=False)
    vb_d = nc.declare_dram_parameter("vb", [1, 390], bf16, isOutput=False)
    pb_d = nc.declare_dram_parameter("pb", [1, C], bf16, isOutput=False)
    pw_d = nc.declare_dram_parameter("pw", [64, NH * C], bf16, isOutput=False)
    rtbl_d = nc.declare_dram_parameter("rtbl", [128, 3 * 8 * 8], bf16, isOutput=False)
    eall_d = nc.declare_dram_parameter("eall", [24, T], bf16, isOutput=False)
    selg_d = nc.declare_dram_parameter("selg", [24, 72], bf16, isOutput=False)
    ones1_d = nc.declare_dram_parameter("ones1", [1, 128], bf16, isOutput=False)
    ones6_d = nc.declare_dram_parameter("ones6", [1, NH], bf16, isOutput=False)
    out_d = nc.declare_dram_parameter("out_w", [4, T, C], f32, isOutput=True)

    with tile.TileContext(nc) as tc, ExitStack() as ctx:
        const = ctx.enter_context(tc.tile_pool(name="const", bufs=1))
        xnat_p = ctx.enter_context(tc.tile_pool(name="xnat", bufs=2))
        xt_p = ctx.enter_context(tc.tile_pool(name="xt", bufs=3))
        vaug_p = ctx.enter_context(tc.tile_pool(name="vaug", bufs=3))
        pp_p = ctx.enter_context(tc.tile_pool(name="pp", bufs=4))
        osb_p = ctx.enter_context(tc.tile_pool(name="osb", bufs=12))
        rd_p = ctx.enter_context(tc.tile_pool(name="rd", bufs=4))
        ost_p = ctx.enter_context(tc.tile_pool(name="ost", bufs=4))
        qkp = ctx.enter_context(tc.tile_pool(name="qkp", bufs=2, space="PSUM"))
        stp = ctx.enter_context(tc.tile_pool(name="stp", bufs=2, space="PSUM"))
        mip = ctx.enter_context(tc.tile_pool(name="mip", bufs=2, space="PSUM"))

        # --- load constants ---
        wq_sb = const.tile([128, NH * 3 * GROW], bf16)
        nc.sync.dma_start(out=wq_sb, in_=wq_d[:, :])
        wk_sb = const.tile([128, NH * 3 * GROW], bf16)
        nc.sync.dma_start(out=wk_sb, in_=wk_d[:, :])
        wv_sb = const.tile([128, 3 * 390], bf16)
        nc.sync.dma_start(out=wv_sb, in_=wv_d[:, :])
        bq_sb = const.tile([128, NH], f32)
        nc.sync.dma_start(out=bq_sb, in_=bq_d[:, :])
        bk_sb = const.tile([128, NH], f32)
        nc.sync.dma_start(out=bk_sb, in_=bk_d[:, :])
        vb_sb = const.tile([1, 390], bf16)
        nc.sync.dma_start(out=vb_sb, in_=vb_d[:, :])
        pb_sb = const.tile([1, C], bf16)
        nc.sync.dma_start(out=pb_sb, in_=pb_d[:, :])
        pw_sb = const.tile([64, NH * C], bf16)
        nc.sync.dma_start(out=pw_sb, in_=pw_d[:, :])
        rtbl_sb = const.tile([128, 3 * 8 * 8], bf16)
        nc.sync.dma_start(out=rtbl_sb, in_=rtbl_d[:, :])
        eall_sb = const.tile([24, T], bf16)
        nc.sync.dma_start(out=eall_sb, in_=eall_d[:, :])
        selg_sb = const.tile([24, 72], bf16)
        nc.sync.dma_start(out=selg_sb, in_=selg_d[:, :])
        ones1_sb = const.tile([1, 128], bf16)
        nc.sync.dma_start(out=ones1_sb, in_=ones1_d[:, :])
        ones6_sb = const.tile([1, NH], bf16)
        nc.sync.dma_start(out=ones6_sb, in_=ones6_d[:, :])

        # persistent per-head tiles: stk (lhsT side: E rows + kT) and rhsq
        # (rhs side: Rel rows + qT)
        stk = [const.tile([128, T], bf16, name=f"stk{h}", tag=f"stk{h}") for h in range(NH)]
        rhsq = [const.tile([128, T], bf16, name=f"rhsq{h}", tag=f"rhsq{h}") for h in range(NH)]

        for w in range(4):
            # --- load pre-transposed x window ---
            xt = xt_p.tile([128, 3, T], bf16)
            for cc in range(3):
                nc.sync.dma_start(out=xt[:, cc, :], in_=x_d[w, :, cc, :])

            # --- v in natural token layout, 65-strided with ones column ---
            va = vaug_p.tile([128, 4, NH, 65], bf16)
            for ct in range(4):
                vp = mip.tile([128, 512], f32, tag="mi", name="vp")
                for cc in range(3):
                    nc.tensor.matmul(
                        vp[:, 0:390],
                        lhsT=xt[:, cc, 128 * ct:128 * ct + 128],
                        rhs=wv_sb[:, 390 * cc:390 * cc + 390],
                        start=(cc == 0), stop=False,
                    )
                nc.tensor.matmul(
                    vp[:, 0:390], lhsT=ones1_sb[0:1, :], rhs=vb_sb[0:1, :],
                    start=False, stop=True,
                )
                nc.vector.tensor_copy(
                    va[:, ct].rearrange("p h x -> p (h x)"), vp[:, 0:390]
                )

            osb_list = []
            for h in range(NH):
                # --- q side: gapped channels + rel terms in the gaps ---
                qp = qkp.tile([128, T], f32, tag="qk", name="qp")
                for cc in range(3):
                    nc.tensor.matmul(
                        qp[0:GROW, :],
                        lhsT=wq_sb[:, (h * 3 + cc) * GROW:(h * 3 + cc + 1) * GROW],
                        rhs=xt[:, cc, :],
                        start=(cc == 0), stop=(cc == 2),
                    )
                # evac 1: biased qT -> rhsq (gap rows become 0)
                nc.vector.tensor_scalar(
                    out=rhsq[h][0:GROW, :], in0=qp[0:GROW, :],
                    scalar1=bq_sb[0:GROW, h:h + 1], scalar2=None, op0=add,
                )
                # rel-pos group matmuls accumulate into the psum gap rows
                rq3 = rhsq[h].rearrange("p (z y x) -> p z y x", z=8, y=8, x=8)
                qp3 = qp.rearrange("p (z y x) -> p z y x", z=8, y=8, x=8)
                for a in range(3):
                    for g in range(8):
                        lhsT = rtbl_sb[0:GROW, (a * 8 + g) * 8:(a * 8 + g + 1) * 8]
                        if a == 0:
                            rhs_ap = rq3[0:GROW, g, :, :]
                            out_ap = qp[0:8, 64 * g:64 * g + 64]
                        elif a == 1:
                            rhs_ap = rq3[0:GROW, :, g, :]
                            out_ap = qp3[32:40, :, g, :]
                        else:
                            rhs_ap = rq3[0:GROW, :, :, g]
                            out_ap = qp3[64:72, :, :, g]
                        # strided psum outs: token-order directly (walrus OK;
                        # CoreSim interp can't model these — validated on hw)
                        nc.tensor.matmul(
                            out_ap, lhsT=lhsT, rhs=rhs_ap,
                            start=(g == 0), stop=(g == 7),
                        )
                # evac 2 (ACT): rewrite chan rows, pick up rel rows
                nc.scalar.activation(
                    out=rhsq[h][0:GROW, :], in_=qp[0:GROW, :],
                    func=Ident, bias=bq_sb[0:GROW, h:h + 1],
                )


                # --- k side: gapped channels + E indicator rows ---
                kp = qkp.tile([128, T], f32, tag="qk", name="kp")
                for cc in range(3):
                    nc.tensor.matmul(
                        kp[0:GROW, :],
                        lhsT=wk_sb[:, (h * 3 + cc) * GROW:(h * 3 + cc + 1) * GROW],
                        rhs=xt[:, cc, :],
                        start=(cc == 0), stop=(cc == 2),
                    )
                    if cc == 0:
                        # E indicator rows into the gap slots (same psum group)
                        nc.tensor.matmul(
                            kp[0:72, :], lhsT=selg_sb[0:24, 0:72],
                            rhs=eall_sb[0:24, :], start=False, stop=False,
                        )
                nc.scalar.activation(
                    out=stk[h][0:GROW, :], in_=kp[0:GROW, :],
                    func=Ident, bias=bk_sb[0:GROW, h:h + 1],
                )

                # --- S^T (+bias) -> exp -> attn@V ---
                ot = mip.tile([128, 512], f32, tag="mi", name="ot")
                for pair in range(2):
                    stt = stp.tile([128, 1024], f32, tag="stt", name="stt")
                    for j in range(2):
                        kc = 2 * pair + j
                        nc.tensor.matmul(
                            stt[:, 512 * j:512 * j + 512],
                            lhsT=stk[h][0:GROW, 128 * kc:128 * kc + 128],
                            rhs=rhsq[h][0:GROW, :],
                            start=True, stop=True,
                        )
                    pp = pp_p.tile([128, 1024], bf16)
                    nc.scalar.activation(out=pp[:, :], in_=stt[:, :], func=Exp)
                    for j in range(2):
                        kc = 2 * pair + j
                        nc.tensor.matmul(
                            ot[0:65, :],
                            lhsT=va[:, kc, h, :],
                            rhs=pp[:, 512 * j:512 * j + 512],
                            start=(kc == 0), stop=(kc == 3),
                        )

                # --- normalize: evacuate OT, recip denom row, matmul-bcast
                # recip into rows 64:128 of the psum bank, multiply ---
                osb_un = osb_p.tile([128, T], bf16, tag="osb_un", name="osb_un")
                nc.vector.tensor_copy(osb_un[0:64, :], ot[0:64, :])
                rdr = rd_p.tile([1, 512], bf16, tag="rdr", name="rdr")
                with nc.allow_low_precision(reason="softmax denom recip in fp16"):
                    nc.vector.reciprocal(rdr[0:1, :], ot[64:65, :])
                nc.tensor.matmul(
                    ot[64:128, :], lhsT=ones1_sb[0:1, 0:64], rhs=rdr[0:1, :],
                    start=True, stop=True,
                )
                osb = osb_p.tile([128, T], bf16)
                nc.vector.tensor_mul(osb[0:64, :], osb_un[0:64, :], ot[64:128, :])
                osb_list.append(osb)

            # --- projection: accumulate heads per q-chunk ---
            for qc in range(4):
                prj = mip.tile([128, 512], f32, tag="mi", name="prj")
                for h in range(NH):
                    nc.tensor.matmul(
                        prj[:, 0:C],
                        lhsT=osb_list[h][0:64, 128 * qc:128 * qc + 128],
                        rhs=pw_sb[0:64, C * h:C * h + C],
                        start=(h == 0), stop=False,
                    )
                nc.tensor.matmul(
                    prj[:, 0:C], lhsT=ones1_sb[0:1, :], rhs=pb_sb[0:1, :],
                    start=False, stop=True,
                )
                ost = ost_p.tile([128, C], f32)
                nc.vector.tensor_copy(ost[:, :], prj[:, 0:C])
                nc.sync.dma_start(
                    out=out_d[w, 128 * qc:128 * qc + 128, :], in_=ost[:, :]
                )

    _fix_multiwait(nc)
    return nc


def _fix_multiwait(nc):
    """Walrus in this container rejects instructions with >1 sync wait.
    Move extra waits onto same-engine NOPs inserted just before."""
    import bass_rust
    import concourse.mybir as mybir

    eng_map = {}
    for eng in (nc.tensor, nc.vector, nc.scalar, nc.gpsimd, nc.sync):
        eng_map[eng.engine] = eng

    f = nc.m.functions[0]
    blocks = list(f.blocks)

    def make_nop(engine_type, wait):
        eng = eng_map[engine_type]
        bi = eng.nop()
        mi = bi.ins
        mi.sync_info = bass_rust.SyncInfo(on_wait=[wait], on_update=[])
        # remove from wherever bass appended it
        for b in blocks:
            bl = b.instructions
            for j in range(len(bl) - 1, -1, -1):
                if bl[j] is mi:
                    del bl[j]
                    return mi
        raise RuntimeError("nop not found after emission")

    for blk in blocks:
        insts = blk.instructions       # live list
        out = []
        changed = False
        for i in insts:
            si = i.sync_info
            if si is not None and len(si.on_wait) > 1:
                waits = list(si.on_wait)
                for w in waits[:-1]:
                    out.append(make_nop(i.engine, w))
                i.sync_info = bass_rust.SyncInfo(
                    on_wait=[waits[-1]], on_update=list(si.on_update)
                )
                changed = True
            out.append(i)
        if changed:
            insts[:] = out


def _host_prep(x, qkv_w, qkv_b, proj_w, proj_b, rel_pos_x, rel_pos_y, rel_pos_z):
    """Build the shared (replicated) device arrays from the raw inputs."""
    qkv_w = np.asarray(qkv_w, np.float32)
    qkv_b = np.asarray(qkv_b, np.float32)
    proj_w = np.asarray(proj_w, np.float32)
    proj_b = np.asarray(proj_b, np.float32)
    rels = [np.asarray(r, np.float32) for r in (rel_pos_x, rel_pos_y, rel_pos_z)]

    cr = np.array(CHAN_ROWS)

    def gapped_w(Wm, scale):
        # Wm [384, 384] -> [128, NH*3*88] lhsT layout
        G = np.zeros((C, NH, GROW), np.float32)
        for h in range(NH):
            G[:, h, cr] = Wm[:, 64 * h:64 * h + 64] * scale
        return np.ascontiguousarray(
            G.reshape(3, 128, NH, GROW).transpose(1, 2, 0, 3).reshape(128, NH * 3 * GROW)
        ).astype(BF16)

    def gapped_b(bm, scale):
        Gb = np.zeros((128, NH), np.float32)
        for h in range(NH):
            Gb[cr, h] = bm[64 * h:64 * h + 64] * scale
        return Gb

    wq_g = gapped_w(qkv_w[:, 0:C], SCALE)
    wk_g = gapped_w(qkv_w[:, C:2 * C], 1.0)
    bq_g = gapped_b(qkv_b[0:C], SCALE)
    bk_g = gapped_b(qkv_b[C:2 * C], 1.0)
    Wv = qkv_w[:, 2 * C:]
    wv_aug = np.zeros((C, 3, NH, 65), np.float32)
    wv_aug[:, :, :, :] = 0.0
    for h in range(NH):
        wv_aug[:, 0, h, 0:64] = 0.0
    Wv3 = Wv.reshape(C, NH, 64)
    wva = np.zeros((C, NH, 65), np.float32)
    wva[:, :, 0:64] = Wv3
    wv = np.ascontiguousarray(
        wva.reshape(3, 128, NH * 65).transpose(1, 0, 2).reshape(128, 3 * 390)
    ).astype(BF16)
    vba = np.zeros((1, NH, 65), np.float32)
    vba[0, :, 0:64] = qkv_b[2 * C:].reshape(NH, 64)
    vba[0, :, 64] = 1.0
    vb = vba.reshape(1, 390).astype(BF16)
    pb = proj_b.reshape(1, C).astype(BF16)
    pw = np.zeros((64, NH * C), np.float32)
    for h in range(NH):
        pw[:, C * h:C * h + C] = proj_w[64 * h:64 * h + 64, :]
    pw = pw.astype(BF16)

    # rel tables: rtbl[chan_row(c), (a*8+g)*8 + dk'] = Ra[g - dk' + 7, c] / SCALE
    # (the rel matmuls consume the already-scaled qT, reference uses unscaled q)
    rtbl = np.zeros((128, 3 * 8 * 8), np.float32)
    for a in range(3):
        Ra = rels[a]  # [15, 64]
        for g in range(8):
            for dk in range(8):
                rtbl[cr, (a * 8 + g) * 8 + dk] = Ra[g - dk + 7, :] / SCALE
    rtbl = rtbl.astype(BF16)

    # E indicators [24, 512]; k = 64*dk + 8*hk + wk
    k_idx = np.arange(T)
    dk, hk, wk = k_idx >> 6, (k_idx >> 3) & 7, k_idx & 7
    eall = np.zeros((24, T), np.float32)
    for cpr in range(8):
        eall[cpr, :] = (dk == cpr)
        eall[8 + cpr, :] = (hk == cpr)
        eall[16 + cpr, :] = (wk == cpr)
    eall = eall.astype(BF16)

    selg = np.zeros((24, 72), np.float32)
    for a in range(3):
        for cpr in range(8):
            selg[8 * a + cpr, 32 * a + cpr] = 1.0
    selg = selg.astype(BF16)

    return dict(
        wq_g=wq_g, wk_g=wk_g, wv=wv, bq_g=bq_g, bk_g=bk_g, vb=vb, pb=pb, pw=pw,
        rtbl=rtbl, eall=eall, selg=selg,
        ones1=np.ones((1, 128), BF16), ones6=np.ones((1, NH), BF16),
    )


LAST_EXEC_NS = None


def kernel(**inputs) -> np.ndarray:
    global LAST_EXEC_NS
    from concourse.bass_utils import run_bass_kernel_spmd

    if "nc" not in _CACHE:
        _CACHE["nc"] = _build_nc()
    nc = _CACHE["nc"]

    x = np.asarray(inputs["x"], np.float32)
    shared = _host_prep(
        x, inputs["qkv_w"], inputs["qkv_b"], inputs["proj_w"], inputs["proj_b"],
        inputs["rel_pos_x"], inputs["rel_pos_y"], inputs["rel_pos_z"],
    )

    # window gather indices within a [2048, C] shard (4 windows x 512 tokens)
    t = np.arange(T)
    z, yy, xx = t >> 6, (t >> 3) & 7, t & 7
    rows_w = np.stack([
        256 * z + 16 * (8 * (w >> 1) + yy) + (8 * (w & 1) + xx) for w in range(4)
    ])  # [4, 512]

    in_maps = []
    for i in range(NCORES):
        b, half = i // 2, i % 2
        m = dict(shared)
        xs = x[b, half * 2048:(half + 1) * 2048, :]          # [2048, C]
        xw = xs[rows_w, :]                                    # [4, 512, C]
        xt4 = xw.transpose(0, 2, 1).reshape(4, 3, 128, T)
        m["xt_sh"] = np.ascontiguousarray(
            xt4.transpose(0, 2, 1, 3)
        ).astype(BF16)                                        # [4, 128, 3, 512]
        in_maps.append(m)

    trace = bool(os.environ.get("KERNEL_TRACE"))
    try:
        res = run_bass_kernel_spmd(
            nc, in_maps, core_ids=list(range(NCORES)), trace=trace,
        )
    except (ModuleNotFoundError, ImportError):
        # NTFF profile hook unavailable in this container - run untraced
        res = run_bass_kernel_spmd(
            nc, in_maps, core_ids=list(range(NCORES)), trace=False,
        )
    LAST_EXEC_NS = res.exec_time_ns

    out = np.empty((B, N, C), np.float32)
    for i in range(NCORES):
        b, half = i // 2, i % 2
        ow = res.results[i]["out_w"]                          # [4, 512, C]
        sh = np.empty((2048, C), np.float32)
        sh[rows_w.reshape(-1), :] = ow.reshape(4 * T, C)
        out[b, half * 2048:(half + 1) * 2048, :] = sh
    return out.reshape(B, N, C)
